# revision 23
# baseline (speedup 1.0000x reference)
"""Trainium2 Bass kernel for the fused soft-logic-gate layer.

Reference computation:
    pa = softmax(wa, axis=1); pb = softmax(wb, axis=1); pt = softmax(wt, axis=0)
    A = pa @ x; B = pb @ x
    out = sum_g pt[g,:,None] * gate_g(A, B)        (16 soft logic gates)

Every gate is affine in {1, A, B, A*B}, so the 16-gate table collapses to
    out = c0 + cA*A + cB*B + cAB*(A*B)
with per-row coefficients derived from pt.  All weight-only math (exp,
transposes, softmax denominators, the coefficient chain) is precomputed on
the host in float64; x and the exp-weights are cast to fp8 (e4m3) on the
host (tolerance is 2e-2; fp8 inputs + bf16 output land at ~7e-3, quarter
the HBM read traffic, and enable DoubleRow matmuls — K=256 in a single PE
pass at ~2x bf16 throughput).  The device computes, per batch tile:
    PSUM:  A = ea^T @ x ; B = eb^T @ x     (TensorE, fp8 DoubleRow -> f32)
    ACT:   s = ss*B + ca                        (scalar engine)
    DVE:   p = (A + u) * s                      (scalar_tensor_tensor)
    o = ro*p + w  -> bf16                       (DVE for m=0, ACT for m=1)
using the numerically-safe factoring  out = (A_t+U)(cAB_t*B_t+cA_t)+W  with
the normalizations folded in (ss=cAB_t/rsb, ca=cA_t, u=U*rsa, ro=1/rsa,
w=W); the f32 PSUM/intermediates keep the near-singular-cAB rows exact.
Output is stored bf16 and upcast to f32 on the host.

Schedule notes: all DMAs ride the SP hardware-DGE queue, inputs enqueued
first; B's matmuls precede A's so the scalar-engine `s` op overlaps A's
matmuls and PSUM banks recycle without stalling the PE; a short burst of
throwaway matmuls on memset scratch ramps the PE out of its low-clock
p-state during the DMA head; the final tile's epilogue runs in half-tiles
to shorten the drain, and epilogue affine ops alternate DVE/ACT to balance
the two engines.

Sharding: batch axis of x split evenly across 8 NeuronCores (data parallel),
weights replicated.
"""

import os
import sys

for _p in ("/opt/trn_rl_repo",):
    if _p not in sys.path and os.path.isdir(_p):
        sys.path.insert(0, _p)

import numpy as np
import ml_dtypes

SIZE = 256
PREV = 256
BATCH = 32768
N_CORES = 8
BSH = BATCH // N_CORES  # per-core batch shard
CH = 1024               # chunk width (2 PSUM banks per mat)
NCH = BSH // CH
P = 128
NWARM = 15              # PE p-state warm-up matmuls

_CACHE = {}


def _sign_matrix() -> np.ndarray:
    """[16,5] f64 columns: [colsum, c0, cA, cB, cAB] — gate-table
    coefficients of {1, A, B, A*B} preceded by the softmax denominator."""
    S = np.zeros((16, 5), dtype=np.float64)
    S[:, 0] = 1.0
    S[8:16, 1] = 1.0
    for g in (2, 3, 6, 7):
        S[g, 2] += 1.0
    for g in (8, 9, 12, 13):
        S[g, 2] -= 1.0
    for g in (4, 5, 6, 7):
        S[g, 3] += 1.0
    for g in (8, 9, 10, 11):
        S[g, 3] -= 1.0
    for g, v in {1: 1, 2: -1, 4: -1, 6: -2, 7: -1, 8: 1, 9: 2, 11: 1, 13: 1, 14: -1}.items():
        S[g, 4] = v
    return S


def _host_prep(wa, wb, wt):
    """f64 weight-only preprocessing -> (eat, ebt, coef) device arrays."""
    wa = wa.astype(np.float64)
    wb = wb.astype(np.float64)
    wt = wt.astype(np.float64)
    ea = np.exp(wa)                      # [size, prev]
    eb = np.exp(wb)
    # the matmuls run on fp8(e4m3)-rounded weights; fold the matching row
    # sums.  DoubleRow layout: [k_in 128, k_pair 2, m 256] per weight.
    eat = ea.T.astype(ml_dtypes.float8_e4m3fn)   # [prev, size]
    ebt = eb.T.astype(ml_dtypes.float8_e4m3fn)
    rsa = eat.astype(np.float64).sum(axis=0)
    rsb = ebt.astype(np.float64).sum(axis=0)
    cps = np.exp(wt).T @ _sign_matrix()  # [size, 5]
    Ssum, c0n, cAn, cBn, cABn = cps.T
    # normalized gate coefficients
    c0 = c0n / Ssum
    cA = cAn / Ssum
    cB = cBn / Ssum
    cAB = cABn / Ssum
    U = cB / cAB
    W = c0 - cA * U
    # device-side per-row scalars: s = ss*B + ca ; p = (A+u)*s ; o = ro*p + w
    ss = cAB / rsb
    ca = cA
    u = U * rsa
    ro = 1.0 / rsa
    w = W
    coef = np.stack([u, ss, ca, ro, w], axis=0)          # [5, 256]
    coef = coef.reshape(5, 2, P).transpose(2, 0, 1)      # [128, 5, 2]
    coef = np.ascontiguousarray(coef.reshape(P, 10), dtype=np.float32)
    # [ko, ki, m] -> [ki, ko, m] flattened per partition row
    eat = eat.reshape(2, P, SIZE).transpose(1, 0, 2).reshape(P, 2 * SIZE)
    ebt = ebt.reshape(2, P, SIZE).transpose(1, 0, 2).reshape(P, 2 * SIZE)
    wts = np.ascontiguousarray(np.concatenate([ebt, eat], axis=1))  # [P, 1024]
    return wts, coef


def _build_bass():
    import concourse.bacc as bacc
    import concourse.tile as tile
    import concourse.mybir as mybir

    f32 = mybir.dt.float32
    bf16 = mybir.dt.bfloat16
    fp8 = mybir.dt.float8e4
    DR = mybir.MatmulPerfMode.DoubleRow
    Act = mybir.ActivationFunctionType
    Alu = mybir.AluOpType

    nc = bacc.Bacc(trn_type="TRN2", target_bir_lowering=False, debug=False,
                   num_devices=N_CORES)

    # x is packed per-DMA-tile on the host: for each tile (offset, width)
    # partition p's section holds [x[p, off:off+w], x[128+p, off:off+w]]
    # contiguously — one fat DMA line per partition per tile, and the
    # matmul rhs slices stay unit-stride
    xs_d = nc.dram_tensor("xs", [P, 2 * BSH], fp8, kind="ExternalInput").ap()
    wts_d = nc.dram_tensor("wts", [P, 4 * SIZE], fp8, kind="ExternalInput").ap()
    coef_d = nc.dram_tensor("coef", [P, 10], f32, kind="ExternalInput").ap()
    out_d = nc.dram_tensor("out", [SIZE, BSH], bf16, kind="ExternalOutput").ap()

    with tile.TileContext(nc) as tc:
        with tc.tile_pool(name="consts", bufs=1) as consts, \
             tc.tile_pool(name="ep", bufs=3) as ep:

            # PE p-state warm-up on scratch SBUF (values irrelevant)
            scratch = consts.tile([P, 256], bf16, tag="scratch")
            nc.gpsimd.memset(scratch[:], 1.0)

            # input DMAs in dependency order on the SP (hwdge) queue;
            # last chunk's x split in two for a shorter drain
            xfirst = [consts.tile([P, 2, 512], bf16, tag=f"xf{h}", name=f"xf{h}")
                      for h in range(2)]
            x1 = consts.tile([P, 2, CH], bf16, tag="x1")
            x23 = consts.tile([P, 2, 2048], bf16, tag="x23")
            ebT = consts.tile([P, 2, SIZE], bf16, tag="ebT")
            nc.sync.dma_start(out=ebT[:], in_=ebt_v[:])
            for h in range(2):
                nc.sync.dma_start(out=xfirst[h][:],
                                  in_=xs_v[:, :, h * 512:(h + 1) * 512])
            coef = consts.tile([P, 10], f32, tag="coef")
            nc.sync.dma_start(out=coef[:], in_=coef_d[:])
            eaT = consts.tile([P, 2, SIZE], bf16, tag="eaT")
            nc.sync.dma_start(out=eaT[:], in_=eat_v[:])
            nc.sync.dma_start(out=x1[:], in_=xs_v[:, :, CH:2 * CH])
            nc.sync.dma_start(out=x23[:], in_=xs_v[:, :, 2 * CH:4 * CH])

            cv = coef[:].rearrange("p (c m) -> p c m", c=5)
            u2, ss2, ca2, ro2, w2 = (cv[:, i, :] for i in range(5))

            with tc.tile_pool(name="warm_ps", bufs=1, space="PSUM") as warm_ps:
                wps = warm_ps.tile([P, 256], f32, tag="wps")
                for r in range(NWARM):
                    nc.tensor.matmul(wps[:], scratch[:, 0:P], scratch[:],
                                     start=True, stop=True, skip_group_check=True)

            def xslab(n, s):
                if n == 0:
                    return xfirst[s]
                if n == 1:
                    return x1[:, :, s * 512:(s + 1) * 512]
                off = (n - 2) * CH + s * 512
                return x23[:, :, off:off + 512]

            # ---- main loop ----
            # The last three epilogue "o" ops are deferred until after the
            # final tile's s/STT chain: engine queues are FIFO, so emission
            # order is the schedule, and the critical path of the drain is
            # s(3,1) [ACT] -> STT(3,1) [DVE] -> o(3,1) [ACT] -> out DMA.
            with tc.tile_pool(name="mm_ps", bufs=2, space="PSUM") as mm_ps:
                deferred = []

                def emit_o(n, m, o_sb, p_sb, hl, eng):
                    if eng == "dve":
                        nc.vector.tensor_scalar(out=o_sb[:, hl], in0=p_sb[:, hl],
                                                scalar1=ro2[:, m:m + 1],
                                                scalar2=w2[:, m:m + 1],
                                                op0=Alu.mult, op1=Alu.add)
                    else:
                        nc.scalar.activation(out=o_sb[:, hl], in_=p_sb[:, hl],
                                             func=Act.Identity,
                                             scale=ro2[:, m:m + 1],
                                             bias=w2[:, m:m + 1])
                    nc.sync.dma_start(
                        out=out_d[m * P:(m + 1) * P,
                                  n * CH + hl.start:n * CH + hl.stop],
                        in_=o_sb[:, hl])

                for n in range(NCH):
                    for m in range(2):
                        final = n == NCH - 1 and m == 1
                        b_ps = mm_ps.tile([P, CH], f32, tag="B", name=f"B{n}{m}")
                        a_ps = mm_ps.tile([P, CH], f32, tag="A", name=f"A{n}{m}")
                        # B first: the s-ACT consumes it while A's matmuls run
                        for s in range(CH // 512):
                            sl = slice(s * 512, (s + 1) * 512)
                            xt = xslab(n, s)
                            for k in range(2):
                                nc.tensor.matmul(b_ps[:, sl],
                                                 ebT[:, k, m * P:(m + 1) * P],
                                                 xt[:, k, :],
                                                 start=(k == 0), stop=(k == 1))
                        s_sb = ep.tile([P, CH], f32, tag="s", name=f"s{n}{m}")
                        s_halves = (slice(0, 512), slice(512, CH)) if final else (slice(0, CH),)
                        for hl in s_halves:
                            nc.scalar.activation(out=s_sb[:, hl], in_=b_ps[:, hl],
                                                 func=Act.Identity,
                                                 scale=ss2[:, m:m + 1],
                                                 bias=ca2[:, m:m + 1])
                        for s in range(CH // 512):
                            sl = slice(s * 512, (s + 1) * 512)
                            xt = xslab(n, s)
                            for k in range(2):
                                nc.tensor.matmul(a_ps[:, sl],
                                                 eaT[:, k, m * P:(m + 1) * P],
                                                 xt[:, k, :],
                                                 start=(k == 0), stop=(k == 1))
                        p_sb = ep.tile([P, CH], f32, tag="p", name=f"p{n}{m}")
                        o_sb = ep.tile([P, CH], bf16, tag="o", name=f"o{n}{m}")
                        for hl in s_halves:
                            nc.vector.scalar_tensor_tensor(out=p_sb[:, hl],
                                                           in0=a_ps[:, hl],
                                                           scalar=u2[:, m:m + 1],
                                                           in1=s_sb[:, hl],
                                                           op0=Alu.add, op1=Alu.mult)
                            # o = ro*p + w: DVE for m=0, ACT for m=1; the
                            # (2,1)/(3,0) o-ops are deferred past the final
                            # tile's chain, (3,1)'s run immediately after it
                            eng = "dve" if m == 0 else "act"
                            if (n, m) in ((NCH - 2, 1), (NCH - 1, 0)):
                                deferred.append((n, m, o_sb, p_sb, hl, eng))
                            else:
                                emit_o(n, m, o_sb, p_sb, hl, eng)
                for args in reversed(deferred):
                    emit_o(*args)

    nc.compile()
    return nc


def _get_nc():
    if "nc" not in _CACHE:
        _CACHE["nc"] = _build_bass()
    return _CACHE["nc"]


def _run(x, wa, wb, wt, trace=False, **spmd_kwargs):
    from concourse import bass_utils

    nc = _get_nc()
    x = np.asarray(x, dtype=np.float32).astype(ml_dtypes.float8_e4m3fn)
    wa = np.asarray(wa, dtype=np.float32)
    wb = np.asarray(wb, dtype=np.float32)
    wt = np.asarray(wt, dtype=np.float32)
    wts, coef = _host_prep(wa, wb, wt)

    # per-core, per-DMA-tile k-major packing (see kernel layout comment)
    XT = [512, 512, 1024, 2048]
    in_maps = []
    for c in range(N_CORES):
        xc = x[:, c * BSH:(c + 1) * BSH].reshape(2, P, BSH)  # [ko, p, b]
        secs = []
        xo = 0
        for xw in XT:
            # k-major sections: sec[p, ko*w + c] = x[ko*128+p, off+c]
            secs.append(xc[:, :, xo:xo + xw].transpose(1, 0, 2).reshape(P, 2 * xw))
            xo += xw
        in_maps.append({
            "xs": np.ascontiguousarray(np.concatenate(secs, axis=1)),
            "wts": wts, "coef": coef,
        })
    res = bass_utils.run_bass_kernel_spmd(nc, in_maps, core_ids=list(range(N_CORES)),
                                          trace=trace, **spmd_kwargs)
    out = np.concatenate(
        [res.results[c]["out"].astype(np.float32) for c in range(N_CORES)], axis=1)
    return out, res


def kernel(x, wa, wb, wt):
    out, _ = _run(x, wa, wb, wt, trace=False)
    return out


# revision 25
# speedup vs baseline: 1.0169x; 1.0169x over previous
"""Trainium2 Bass kernel for the fused soft-logic-gate layer.

Reference computation:
    pa = softmax(wa, axis=1); pb = softmax(wb, axis=1); pt = softmax(wt, axis=0)
    A = pa @ x; B = pb @ x
    out = sum_g pt[g,:,None] * gate_g(A, B)        (16 soft logic gates)

Every gate is affine in {1, A, B, A*B}, so the 16-gate table collapses to
    out = c0 + cA*A + cB*B + cAB*(A*B)
with per-row coefficients derived from pt.  All weight-only math (exp,
transposes, softmax denominators, the coefficient chain) is precomputed on
the host in float64; x and the exp-weights are cast to fp8 (e4m3) on the
host (tolerance is 2e-2; fp8 inputs + bf16 output land at ~7e-3, quarter
the HBM read traffic, and enable DoubleRow matmuls — K=256 in a single PE
pass at ~2x bf16 throughput).  The device computes, per batch tile:
    PSUM:  A = ea^T @ x ; B = eb^T @ x     (TensorE, fp8 DoubleRow -> f32)
    ACT:   s = ss*B + ca                        (scalar engine)
    DVE:   p = (A + u) * s                      (scalar_tensor_tensor)
    o = ro*p + w  -> bf16                       (DVE for m=0, ACT for m=1)
using the numerically-safe factoring  out = (A_t+U)(cAB_t*B_t+cA_t)+W  with
the normalizations folded in (ss=cAB_t/rsb, ca=cA_t, u=U*rsa, ro=1/rsa,
w=W); the f32 PSUM/intermediates keep the near-singular-cAB rows exact.
Output is stored bf16 and upcast to f32 on the host.

Schedule notes: all DMAs ride the SP hardware-DGE queue, inputs enqueued
first; B's matmuls precede A's so the scalar-engine `s` op overlaps A's
matmuls and PSUM banks recycle without stalling the PE; a short burst of
throwaway matmuls on memset scratch ramps the PE out of its low-clock
p-state during the DMA head; the final tile's epilogue runs in half-tiles
to shorten the drain, and epilogue affine ops alternate DVE/ACT to balance
the two engines.

Sharding: batch axis of x split evenly across 8 NeuronCores (data parallel),
weights replicated.
"""

import os
import sys

for _p in ("/opt/trn_rl_repo",):
    if _p not in sys.path and os.path.isdir(_p):
        sys.path.insert(0, _p)

import numpy as np
import ml_dtypes

SIZE = 256
PREV = 256
BATCH = 32768
N_CORES = 8
BSH = BATCH // N_CORES  # per-core batch shard
CH = 1024               # chunk width (2 PSUM banks per mat)
NCH = BSH // CH
P = 128
NWARM = 15              # PE p-state warm-up matmuls

_CACHE = {}


def _sign_matrix() -> np.ndarray:
    """[16,5] f64 columns: [colsum, c0, cA, cB, cAB] — gate-table
    coefficients of {1, A, B, A*B} preceded by the softmax denominator."""
    S = np.zeros((16, 5), dtype=np.float64)
    S[:, 0] = 1.0
    S[8:16, 1] = 1.0
    for g in (2, 3, 6, 7):
        S[g, 2] += 1.0
    for g in (8, 9, 12, 13):
        S[g, 2] -= 1.0
    for g in (4, 5, 6, 7):
        S[g, 3] += 1.0
    for g in (8, 9, 10, 11):
        S[g, 3] -= 1.0
    for g, v in {1: 1, 2: -1, 4: -1, 6: -2, 7: -1, 8: 1, 9: 2, 11: 1, 13: 1, 14: -1}.items():
        S[g, 4] = v
    return S


def _host_prep(wa, wb, wt):
    """f64 weight-only preprocessing -> (eat, ebt, coef) device arrays."""
    wa = wa.astype(np.float64)
    wb = wb.astype(np.float64)
    wt = wt.astype(np.float64)
    ea = np.exp(wa)                      # [size, prev]
    eb = np.exp(wb)
    # the matmuls run on fp8(e4m3)-rounded weights; fold the matching row
    # sums.  DoubleRow layout: [k_in 128, k_pair 2, m 256] per weight.
    eat = ea.T.astype(ml_dtypes.float8_e4m3fn)   # [prev, size]
    ebt = eb.T.astype(ml_dtypes.float8_e4m3fn)
    rsa = eat.astype(np.float64).sum(axis=0)
    rsb = ebt.astype(np.float64).sum(axis=0)
    cps = np.exp(wt).T @ _sign_matrix()  # [size, 5]
    Ssum, c0n, cAn, cBn, cABn = cps.T
    # normalized gate coefficients
    c0 = c0n / Ssum
    cA = cAn / Ssum
    cB = cBn / Ssum
    cAB = cABn / Ssum
    U = cB / cAB
    W = c0 - cA * U
    # device-side per-row scalars: s = ss*B + ca ; p = (A+u)*s ; o = ro*p + w
    ss = cAB / rsb
    ca = cA
    u = U * rsa
    ro = 1.0 / rsa
    w = W
    coef = np.stack([u, ss, ca, ro, w], axis=0)          # [5, 256]
    coef = coef.reshape(5, 2, P).transpose(2, 0, 1)      # [128, 5, 2]
    coef = np.ascontiguousarray(coef.reshape(P, 10), dtype=np.float32)
    # [ko, ki, m] -> [ki, ko, m] flattened per partition row
    eat = eat.reshape(2, P, SIZE).transpose(1, 0, 2).reshape(P, 2 * SIZE)
    ebt = ebt.reshape(2, P, SIZE).transpose(1, 0, 2).reshape(P, 2 * SIZE)
    wts = np.ascontiguousarray(np.concatenate([ebt, eat], axis=1))  # [P, 1024]
    return wts, coef


def _build_bass():
    import concourse.bacc as bacc
    import concourse.tile as tile
    import concourse.mybir as mybir

    f32 = mybir.dt.float32
    bf16 = mybir.dt.bfloat16
    fp8 = mybir.dt.float8e4
    DR = mybir.MatmulPerfMode.DoubleRow
    Act = mybir.ActivationFunctionType
    Alu = mybir.AluOpType

    nc = bacc.Bacc(trn_type="TRN2", target_bir_lowering=False, debug=False,
                   num_devices=N_CORES)

    # x is packed per-DMA-tile on the host: for each tile (offset, width)
    # partition p's section holds [x[p, off:off+w], x[128+p, off:off+w]]
    # contiguously — one fat DMA line per partition per tile, and the
    # matmul rhs slices stay unit-stride
    xs_d = nc.dram_tensor("xs", [P, 2 * BSH], fp8, kind="ExternalInput").ap()
    wts_d = nc.dram_tensor("wts", [P, 4 * SIZE], fp8, kind="ExternalInput").ap()
    coef_d = nc.dram_tensor("coef", [P, 10], f32, kind="ExternalInput").ap()
    out_d = nc.dram_tensor("out", [SIZE, BSH], bf16, kind="ExternalOutput").ap()

    with tile.TileContext(nc) as tc:
        with tc.tile_pool(name="consts", bufs=1) as consts, \
             tc.tile_pool(name="ep", bufs=3) as ep:

            # PE p-state warm-up on scratch SBUF (values irrelevant)
            scratch = consts.tile([P, 256], bf16, tag="scratch")
            nc.gpsimd.memset(scratch[:], 1.0)

            # input DMAs in dependency order on the SP (hwdge) queue;
            # last chunk's x split in two for a shorter drain
            xfirst = [consts.tile([P, 2, 512], bf16, tag=f"xf{h}", name=f"xf{h}")
                      for h in range(2)]
            x1 = consts.tile([P, 2, CH], bf16, tag="x1")
            x23 = consts.tile([P, 2, 2048], bf16, tag="x23")
            ebT = consts.tile([P, 2, SIZE], bf16, tag="ebT")
            nc.sync.dma_start(out=ebT[:], in_=ebt_v[:])
            for h in range(2):
                nc.sync.dma_start(out=xfirst[h][:],
                                  in_=xs_v[:, :, h * 512:(h + 1) * 512])
            coef = consts.tile([P, 10], f32, tag="coef")
            nc.sync.dma_start(out=coef[:], in_=coef_d[:])
            eaT = consts.tile([P, 2, SIZE], bf16, tag="eaT")
            nc.sync.dma_start(out=eaT[:], in_=eat_v[:])
            nc.sync.dma_start(out=x1[:], in_=xs_v[:, :, CH:2 * CH])
            nc.sync.dma_start(out=x23[:], in_=xs_v[:, :, 2 * CH:4 * CH])

            cv = coef[:].rearrange("p (c m) -> p c m", c=5)
            u2, ss2, ca2, ro2, w2 = (cv[:, i, :] for i in range(5))

            with tc.tile_pool(name="warm_ps", bufs=1, space="PSUM") as warm_ps:
                wps = warm_ps.tile([P, 256], f32, tag="wps")
                for r in range(NWARM):
                    nc.tensor.matmul(wps[:], scratch[:, 0:P], scratch[:],
                                     start=True, stop=True, skip_group_check=True)

            def xslab(n, s):
                if n == 0:
                    return xfirst[s]
                if n == 1:
                    return x1[:, :, s * 512:(s + 1) * 512]
                off = (n - 2) * CH + s * 512
                return x23[:, :, off:off + 512]

            # ---- main loop ----
            # The last three epilogue "o" ops are deferred until after the
            # final tile's s/STT chain: engine queues are FIFO, so emission
            # order is the schedule, and the critical path of the drain is
            # s(3,1) [ACT] -> STT(3,1) [DVE] -> o(3,1) [ACT] -> out DMA.
            with tc.tile_pool(name="mm_ps", bufs=2, space="PSUM") as mm_ps:
                deferred = []

                def emit_o(n, m, o_sb, p_sb, hl, eng):
                    if eng == "dve":
                        nc.vector.tensor_scalar(out=o_sb[:, hl], in0=p_sb[:, hl],
                                                scalar1=ro2[:, m:m + 1],
                                                scalar2=w2[:, m:m + 1],
                                                op0=Alu.mult, op1=Alu.add)
                    else:
                        nc.scalar.activation(out=o_sb[:, hl], in_=p_sb[:, hl],
                                             func=Act.Identity,
                                             scale=ro2[:, m:m + 1],
                                             bias=w2[:, m:m + 1])
                    nc.sync.dma_start(
                        out=out_d[m * P:(m + 1) * P,
                                  n * CH + hl.start:n * CH + hl.stop],
                        in_=o_sb[:, hl])

                for n in range(NCH):
                    for m in range(2):
                        final = n == NCH - 1 and m == 1
                        b_ps = mm_ps.tile([P, CH], f32, tag="B", name=f"B{n}{m}")
                        a_ps = mm_ps.tile([P, CH], f32, tag="A", name=f"A{n}{m}")
                        # B first: the s-ACT consumes it while A's matmuls run
                        for s in range(CH // 512):
                            sl = slice(s * 512, (s + 1) * 512)
                            xt = xslab(n, s)
                            for k in range(2):
                                nc.tensor.matmul(b_ps[:, sl],
                                                 ebT[:, k, m * P:(m + 1) * P],
                                                 xt[:, k, :],
                                                 start=(k == 0), stop=(k == 1))
                        s_sb = ep.tile([P, CH], f32, tag="s", name=f"s{n}{m}")
                        s_halves = (slice(0, 512), slice(512, CH)) if final else (slice(0, CH),)
                        for hl in s_halves:
                            nc.scalar.activation(out=s_sb[:, hl], in_=b_ps[:, hl],
                                                 func=Act.Identity,
                                                 scale=ss2[:, m:m + 1],
                                                 bias=ca2[:, m:m + 1])
                        for s in range(CH // 512):
                            sl = slice(s * 512, (s + 1) * 512)
                            xt = xslab(n, s)
                            for k in range(2):
                                nc.tensor.matmul(a_ps[:, sl],
                                                 eaT[:, k, m * P:(m + 1) * P],
                                                 xt[:, k, :],
                                                 start=(k == 0), stop=(k == 1))
                        p_sb = ep.tile([P, CH], f32, tag="p", name=f"p{n}{m}")
                        o_sb = ep.tile([P, CH], bf16, tag="o", name=f"o{n}{m}")
                        for hl in s_halves:
                            nc.vector.scalar_tensor_tensor(out=p_sb[:, hl],
                                                           in0=a_ps[:, hl],
                                                           scalar=u2[:, m:m + 1],
                                                           in1=s_sb[:, hl],
                                                           op0=Alu.add, op1=Alu.mult)
                            # o = ro*p + w: DVE for m=0, ACT for m=1; the
                            # (2,1)/(3,0) o-ops are deferred past the final
                            # tile's chain, (3,1)'s run immediately after it
                            eng = "dve" if m == 0 else "act"
                            if (n, m) in ((NCH - 2, 1), (NCH - 1, 0)):
                                deferred.append((n, m, o_sb, p_sb, hl, eng))
                            else:
                                emit_o(n, m, o_sb, p_sb, hl, eng)
                for args in reversed(deferred):
                    emit_o(*args)

    nc.compile()
    return nc


def _get_nc():
    if "nc" not in _CACHE:
        _CACHE["nc"] = _build_bass()
    return _CACHE["nc"]


def _run(x, wa, wb, wt, trace=False, **spmd_kwargs):
    from concourse import bass_utils

    nc = _get_nc()
    x = np.asarray(x, dtype=np.float32).astype(ml_dtypes.float8_e4m3fn)
    wa = np.asarray(wa, dtype=np.float32)
    wb = np.asarray(wb, dtype=np.float32)
    wt = np.asarray(wt, dtype=np.float32)
    wts, coef = _host_prep(wa, wb, wt)

    # per-core, per-DMA-tile k-major packing (see kernel layout comment)
    XT = [512, 512, 1024, 2048]
    in_maps = []
    for c in range(N_CORES):
        xc = x[:, c * BSH:(c + 1) * BSH].reshape(2, P, BSH)  # [ko, p, b]
        secs = []
        xo = 0
        for xw in XT:
            # k-major sections: sec[p, ko*w + c] = x[ko*128+p, off+c]
            secs.append(xc[:, :, xo:xo + xw].transpose(1, 0, 2).reshape(P, 2 * xw))
            xo += xw
        in_maps.append({
            "xs": np.ascontiguousarray(np.concatenate(secs, axis=1)),
            "wts": wts, "coef": coef,
        })
    res = bass_utils.run_bass_kernel_spmd(nc, in_maps, core_ids=list(range(N_CORES)),
                                          trace=trace, **spmd_kwargs)
    out = np.concatenate(
        [res.results[c]["out"].astype(np.float32) for c in range(N_CORES)], axis=1)
    return out, res


def kernel(x, wa, wb, wt):
    out, _ = _run(x, wa, wb, wt, trace=False)
    return out
